# revision 36
# baseline (speedup 1.0000x reference)
"""Bahdanau-attention kernel for 8 Trainium2 NeuronCores (SPMD, batch-sharded).

Algorithm: scores[t,s] = sum_h v_h * tanh(D[h,t] + E[h,s]) via a free-frequency
sine expansion  tanh(x) ~= sum_k b_k sin(w_k x)  (F=4, Gaussian-weighted fit),
factored through the angle-addition formula into 2F PSUM-accumulating bf16
matmuls over sin/cos features of uD = W2^T dec^T and uE = W1^T enc^T (each
computed once as a bf16 matmul into its own PSUM tensor -- separate tensors so
the two PSUM->SBUF copies, DVE for e and ACT for d, run in parallel; Tile
serializes cross-engine access within a single PSUM tensor).

Per-frequency features (wide [128,768] e||d instructions; bf16 out):
  k=0:  |w0*u| <= ~1.6 fits the Sin LUT domain (~+-3.55): fS = Sin(w0*u),
        fC = Sin(w0*u + pi/2). No range reduction.
  k=1:  fS = Sin(w1*u) (rare |arg|>3.55 inputs clamp; error negligible),
        fC = Sin(-w1*|u| + pi/2), |u| via one sign-bit-mask tensor_scalar.
  k=2,3: magic-constant range reduction on DVE: v = u*s_k; i = (v+M)-M;
        a = v-i (TT); b = |a| via bitwise-AND sign mask; then Sin(2pi*a)
        and Sin(-2pi*b + pi/2).
v*b_k folds into the decoder features (bf16 per-partition tensor_scalar), so
each score needs only 4 accumulating matmuls per frequency. The encoder
padding mask enters PSUM as a -1e30 seed via K=1 rank-1 bf16 matmuls emitted
during the input-DMA shadow; softmax runs without max-shift (bf16 exp, f32
accum_out row sums); the decoder mask folds into the 1/sum scale; output is
stored bf16 and upcast on host. Inputs arrive via two parallel-queue packed
bf16 DMAs -- [W1|encT] first since it feeds the critical uE->copy->chain
path, then [W2|decT|vb:dm] (the f32 vb/dm rides as raw bits, bitcast on
device, with explicit add_dep edges on its readers) -- plus a tiny mask-row
DMA. Exp/Sin ACT table sets are preloaded via dummy activations before the
data arrives; transposes/casts are host-side layout prep.
"""
import os
import sys

import numpy as np

if "/opt/trn_rl_repo" not in sys.path:
    sys.path.insert(0, "/opt/trn_rl_repo")

S, T, B, H = 512, 256, 8, 128
F = 4
OMEGA = np.array(
    [0.28378837870584145, 0.8460440611349613,
     1.5065032153851483, 2.6337314948021557], dtype=np.float64
)
BK = np.array(
    [1.2424371140304091, 0.31247076876828433,
     0.16201975442700692, 0.045413278208112536], dtype=np.float64
)
TWO_PI = float(2.0 * np.pi)
PI = float(np.pi)
HALF_PI = float(0.5 * np.pi)
NEG_BIG = -1.0e30

_CACHE = {}
LAST_EXEC_NS = None


def _try_install_trace_hook():
    """Best-effort NTFF profile hook for axon (used only when tracing)."""
    try:
        import contextlib
        import ctypes
        import types

        if "antenv.axon_hooks" in sys.modules:
            return
        lib = ctypes.CDLL("/opt/axon/libaxon_pjrt.so")
        if not hasattr(lib, "axon_start_nrt_profile"):
            return
        lib.axon_start_nrt_profile.argtypes = [
            ctypes.POINTER(ctypes.c_int64),
            ctypes.c_size_t,
        ]
        lib.axon_start_nrt_profile.restype = ctypes.c_int64
        lib.axon_stop_nrt_profile.argtypes = [ctypes.c_char_p]
        lib.axon_stop_nrt_profile.restype = ctypes.c_int64

        @contextlib.contextmanager
        def _hook(output_dir, device_ids):
            import jax

            jax.devices()
            if device_ids:
                ids = (ctypes.c_int64 * len(device_ids))(*device_ids)
                rc = lib.axon_start_nrt_profile(ids, len(device_ids))
            else:
                rc = lib.axon_start_nrt_profile(None, 0)
            if rc != 0:
                raise RuntimeError(f"axon_start_nrt_profile rc={rc}")
            try:
                yield
            finally:
                n = lib.axon_stop_nrt_profile(str(output_dir).encode())
                if n < 0:
                    raise RuntimeError(f"axon_stop_nrt_profile rc={n}")

        mod = types.ModuleType("antenv.axon_hooks")
        _h = _hook

        def set_axon_ntff_profile_hook(h):
            pass

        def get_axon_ntff_profile_hook():
            return _h

        mod.set_axon_ntff_profile_hook = set_axon_ntff_profile_hook
        mod.get_axon_ntff_profile_hook = get_axon_ntff_profile_hook
        sys.modules["antenv.axon_hooks"] = mod
        import antenv

        antenv.axon_hooks = mod
    except Exception:
        pass


def _build():
    if "nc" in _CACHE:
        return _CACHE["nc"]
    import concourse.bacc as bacc
    import concourse.tile as tile
    from concourse.tile import add_dep_helper
    import concourse.mybir as mybir

    F32 = mybir.dt.float32
    U32 = mybir.dt.uint32
    BF16 = mybir.dt.bfloat16
    FP16 = mybir.dt.float16
    AF = mybir.ActivationFunctionType
    AL = mybir.AluOpType

    SCAL = [float(w / (2.0 * np.pi)) for w in OMEGA]

    nc = bacc.Bacc("TRN2", target_bir_lowering=False, debug=False, num_devices=8)

    PK1C = (H + T) + 2 * (F + 2)
    pk2_d = nc.dram_tensor("pack2", [H, H + S], BF16, kind="ExternalInput")
    pk1_d = nc.dram_tensor("pack1", [H, PK1C], BF16, kind="ExternalInput")
    em_d = nc.dram_tensor("encmask", [1, S], BF16, kind="ExternalInput")
    out_d = nc.dram_tensor("out", [T, S], BF16, kind="ExternalOutput")

    with tile.TileContext(nc) as tc:
        with (
            tc.tile_pool(name="cst", bufs=1) as cst,
            tc.tile_pool(name="wrk", bufs=1) as wrk,
            tc.tile_pool(name="ps", bufs=1, space="PSUM") as psp,
        ):
            # ---- inputs via two parallel-queue DMAs: the e-side pack
            # ([W1|encT], feeds the critical uE->copy->chain path) rings
            # first, then [W2|decT|vb:dm-bits], then the tiny mask row ----
            with nc.named_scope("dma_in"):
                pk2_sb = cst.tile([H, H + S], BF16)
                nc.sync.dma_start(pk2_sb[:], pk2_d[:])
                pk1_sb = cst.tile([H, PK1C], BF16)
                pk_dma = nc.sync.dma_start(pk1_sb[:], pk1_d[:])
                em_sb = cst.tile([1, S], BF16)
                nc.sync.dma_start(em_sb[:], em_d[:])

            p1 = pk1_sb[:, 0:H + T]
            p2 = pk2_sb[:]
            p3 = pk1_sb[:, H + T:PK1C].bitcast(F32)

            ones_sb = cst.tile([1, H], BF16)
            nc.gpsimd.memset(ones_sb[:], 1.0)
            hp_sb = cst.tile([128, 1], F32)
            nc.gpsimd.memset(hp_sb[:], HALF_PI)

            # Preload the Exp table set into the second table slot while
            # ACT is idle, so the softmax needs no mid-stream table load.
            scr = wrk.tile([128, 1], F32, name="scr")
            nc.scalar.activation(scr[:], hp_sb[:], AF.Exp)
            scr2 = wrk.tile([128, 1], F32, name="scr2")
            nc.scalar.activation(scr2[:], hp_sb[:], AF.Sin)

            # ---- u in PSUM as TWO tensors (bank-disjoint) so the two
            # PSUM->SBUF copies (DVE: e, ACT: d) run in parallel; Tile
            # serializes cross-engine access within one PSUM tensor ----
            uE_ps = psp.tile([128, S], F32, tag="upsE")
            uD_ps = psp.tile([128, T], F32, tag="upsD")
            with nc.named_scope("u_mm"):
                nc.tensor.matmul(
                    uE_ps[:], p2[:, 0:H], p2[:, H:], start=True, stop=True)
                nc.tensor.matmul(
                    uD_ps[:], p1[:, 0:H], p1[:, H:], start=True, stop=True)
            u_sb = wrk.tile([128, 768], F32, name="u_sb")
            uA = u_sb[:, 0:S + T]
            with nc.named_scope("u_copy"):
                nc.vector.tensor_scalar_mul(u_sb[:, 0:S], uE_ps[:], 1.0)
                nc.scalar.copy(u_sb[:, S:], uD_ps[:])

            sc = []
            # score PSUM seeded with -1e30 encoder mask
            for tb in range(2):
                sc_tile = psp.tile([128, S], F32, tag=f"sc{tb}")
                sc.append(sc_tile)
                with nc.named_scope(f"mask_{tb}"):
                    nc.tensor.matmul(
                        sc_tile[:], ones_sb[:], em_sb[:],
                        start=True, stop=False, skip_group_check=True,
                    )


            W = S + T  # 768: all feature tiles are [e(512) | d(256)]
            M32 = float(1.5 * 2**23)

            def folds(k, fS, fC):
                """v*b_k folds into d-side features -> new [128,T] bf16 tiles.
                Explicit dep on the pack DMA: p3 is a bitcast view and must
                not race ahead of the transfer."""
                with nc.named_scope(f"vfold_{k}"):
                    fSdv = wrk.tile([128, T], BF16, name=f"fSdv{k}")
                    i1 = nc.vector.tensor_scalar_mul(
                        fSdv[:], fS[:, S:], p3[:, k:k + 1])
                    fCdv = wrk.tile([128, T], BF16, name=f"fCdv{k}")
                    i2 = nc.vector.tensor_scalar_mul(
                        fCdv[:], fC[:, S:], p3[:, k:k + 1])
                add_dep_helper(i1.ins, pk_dma.ins, reason="p3 bitcast read after DMA")
                add_dep_helper(i2.ins, pk_dma.ins, reason="p3 bitcast read after DMA")
                return fSdv, fCdv

            def scores(k, fSdv, fCdv, fS, fC):
                with nc.named_scope(f"scores_{k}"):
                    last = k == F - 1
                    for tb in range(2):
                        dsl = slice(tb * 128, (tb + 1) * 128)
                        nc.tensor.matmul(
                            sc[tb][:], fSdv[:, dsl], fC[:, 0:S],
                            start=False, stop=False, skip_group_check=True,
                        )
                        nc.tensor.matmul(
                            sc[tb][:], fCdv[:, dsl], fS[:, 0:S],
                            start=False, stop=last, skip_group_check=True,
                        )

            # ---- ACT stream (all inputs from SBUF) ----
            with nc.named_scope("sin_0"):
                fS0 = wrk.tile([128, W], BF16, name="fS0")
                nc.scalar.activation(fS0[:], uA, AF.Sin, scale=float(OMEGA[0]))
                fC0 = wrk.tile([128, W], BF16, name="fC0")
                nc.scalar.activation(
                    fC0[:], uA, AF.Sin, bias=hp_sb[:], scale=float(OMEGA[0]))

            absu = wrk.tile([128, W], F32, name="absu")
            absu_i = nc.vector.tensor_scalar(
                absu[:].bitcast(U32), uA.bitcast(U32), 0x7FFFFFFF, None,
                AL.bitwise_and)

            with nc.named_scope("sin_1"):
                fS1 = wrk.tile([128, W], BF16, name="fS1")
                nc.scalar.activation(fS1[:], uA, AF.Sin, scale=float(OMEGA[1]))
                fC1 = wrk.tile([128, W], BF16, name="fC1")
                fC1_i = nc.scalar.activation(
                    fC1[:], absu[:], AF.Sin, bias=hp_sb[:],
                    scale=float(-OMEGA[1]))
            add_dep_helper(fC1_i.ins, absu_i.ins, reason="fC1 reads sign-masked absu")

            # ---- k=2,3: magic-constant range reduction (all DVE), with
            # folds interleaved so PE gets its lhs operands early; |x| via
            # sign-bit masking (single-source tensor_scalar, 2x mode) ----
            parts = {}
            for k in (2, 3):
                with nc.named_scope(f"red_{k}"):
                    vv = wrk.tile([128, W], F32, name=f"v{k}")
                    nc.vector.tensor_scalar_mul(vv[:], uA, SCAL[k])
                    ii = wrk.tile([128, W], F32, name=f"i{k}")
                    nc.vector.tensor_scalar(
                        ii[:], vv[:], M32, M32, AL.add, AL.subtract)
                    aa = wrk.tile([128, W], F32, name=f"a{k}")
                    nc.vector.tensor_tensor(aa[:], vv[:], ii[:], AL.subtract)
                    bb = wrk.tile([128, W], F32, name=f"b{k}")
                    bb_i = nc.vector.tensor_scalar(
                        bb[:].bitcast(U32), aa[:].bitcast(U32), 0x7FFFFFFF,
                        None, AL.bitwise_and)
                with nc.named_scope(f"sin_{k}"):
                    fSk = wrk.tile([128, W], BF16, name=f"fS{k}")
                    nc.scalar.activation(fSk[:], aa[:], AF.Sin, scale=TWO_PI)
                    fCk = wrk.tile([128, W], BF16, name=f"fC{k}")
                    fCk_i = nc.scalar.activation(
                        fCk[:], bb[:], AF.Sin, bias=hp_sb[:], scale=-TWO_PI)
                add_dep_helper(fCk_i.ins, bb_i.ins, reason="fC reads sign-masked b")
                parts[k] = (fSk, fCk)
                if k == 2:
                    fS0v, fC0v = folds(0, fS0, fC0)
                    scores(0, fS0v, fC0v, fS0, fC0)
            fS1v, fC1v = folds(1, fS1, fC1)
            scores(1, fS1v, fC1v, fS1, fC1)
            for k in (2, 3):
                fSk, fCk = parts[k]
                fSkv, fCkv = folds(k, fSk, fCk)
                scores(k, fSkv, fCkv, fSk, fCk)

            # ---- softmax + decoder-mask scale + store; the two row-block
            # chains are interleaved so tb1's accum/recip overlaps tb0's
            # scale+store ----
            ex, rs, fac, ot = {}, {}, {}, {}
            for tb in range(2):
                with nc.named_scope(f"exp_{tb}"):
                    ex[tb] = wrk.tile([128, S], BF16, name=f"ex{tb}")
                    rs[tb] = wrk.tile([128, 1], F32, name=f"rs{tb}")
                    nc.scalar.activation(
                        ex[tb][:], sc[tb][:], AF.Exp, accum_out=rs[tb][:])
            for tb in range(2):
                with nc.named_scope(f"scale_{tb}"):
                    ri = wrk.tile([128, 1], F32, name=f"ri{tb}")
                    nc.vector.reciprocal(ri[:], rs[tb][:])
                    fac[tb] = wrk.tile([128, 1], F32, name=f"fac{tb}")
                    fac_i = nc.vector.tensor_tensor(
                        fac[tb][:], ri[:], p3[:, F + tb:F + tb + 1],
                        mybir.AluOpType.mult)
                    add_dep_helper(fac_i.ins, pk_dma.ins, reason="p3 bitcast read")
                    ot[tb] = wrk.tile([128, S], BF16, name=f"ot{tb}")
                    nc.vector.tensor_scalar_mul(ot[tb][:], ex[tb][:], fac[tb][:])
                    # ring tb1's doorbell from idle GPSIMD so it never
                    # queues behind tb0's on the Sync engine
                    ring = nc.gpsimd if tb == 1 else nc.sync
                    ring.dma_start(out_d[tb * 128:(tb + 1) * 128, :], ot[tb][:])

    nc.compile()
    _CACHE["nc"] = nc
    return nc


def kernel(encoder_output, decoder_output, W1, W2, v, enc_lens, dec_lens):
    global LAST_EXEC_NS
    from concourse.bass_utils import run_bass_kernel_spmd
    import ml_dtypes

    BF = ml_dtypes.bfloat16
    enc = np.asarray(encoder_output, dtype=np.float32)
    dec = np.asarray(decoder_output, dtype=np.float32)
    W1 = np.asarray(W1, dtype=np.float32)
    W2 = np.asarray(W2, dtype=np.float32)
    v = np.asarray(v, dtype=np.float32)
    enc_lens = np.asarray(enc_lens)
    dec_lens = np.asarray(dec_lens)

    vb = (v[:, None].astype(np.float64) * BK[None, :]).astype(np.float32)  # (H,F)

    in_maps = []
    for b in range(B):
        p1 = np.concatenate([W2, dec[:, b, :].T], axis=1).astype(BF)
        p2 = np.ascontiguousarray(
            np.concatenate([W1, enc[:, b, :].T], axis=1).astype(BF))
        dm = (np.arange(T) < int(dec_lens[b])).astype(np.float32)
        p3 = np.ascontiguousarray(
            np.concatenate([vb, dm.reshape(H, 2, order="F")], axis=1),
            dtype=np.float32)
        p3_bits = p3.view(np.uint16).view(BF)  # raw f32 bits as bf16 pairs
        pack1 = np.ascontiguousarray(np.concatenate([p1, p3_bits], axis=1))
        em = np.where(
            np.arange(S)[None, :] < int(enc_lens[b]), 0.0, NEG_BIG
        ).astype(BF)
        in_maps.append({"pack1": pack1, "pack2": p2, "encmask": em})

    trace = os.environ.get("KERNEL_TRACE", "0") == "1"
    if trace:
        _try_install_trace_hook()
    nc = _build()
    ncores = int(os.environ.get("KERNEL_CORES", str(B)))
    res = run_bass_kernel_spmd(nc, in_maps[:ncores], core_ids=list(range(ncores)), trace=trace)
    if trace:
        LAST_EXEC_NS = res.exec_time_ns
        _CACHE["last_res"] = res

    out = np.zeros((T, B, S), dtype=np.float32)
    for b in range(ncores):
        out[:, b, :] = np.asarray(res.results[b]["out"], dtype=np.float32)
    return out


# revision 37
# speedup vs baseline: 1.0558x; 1.0558x over previous
"""Bahdanau-attention kernel for 8 Trainium2 NeuronCores (SPMD, batch-sharded).

Algorithm: scores[t,s] = sum_h v_h * tanh(D[h,t] + E[h,s]) via a free-frequency
sine expansion  tanh(x) ~= sum_k b_k sin(w_k x)  (F=4, Gaussian-weighted fit),
factored through the angle-addition formula into 2F PSUM-accumulating bf16
matmuls over sin/cos features of uD = W2^T dec^T and uE = W1^T enc^T (each
computed once as a bf16 matmul into its own PSUM tensor -- separate tensors so
the two PSUM->SBUF copies, DVE for e and ACT for d, run in parallel; Tile
serializes cross-engine access within a single PSUM tensor).

Per-frequency features (wide [128,768] e||d instructions; bf16 out):
  k=0:  |w0*u| <= ~1.6 fits the Sin LUT domain (~+-3.55): fS = Sin(w0*u),
        fC = Sin(w0*u + pi/2). No range reduction.
  k=1:  fS = Sin(w1*u) (rare |arg|>3.55 inputs clamp; error negligible),
        fC = Sin(-w1*|u| + pi/2), |u| via one sign-bit-mask tensor_scalar.
  k=2,3: magic-constant range reduction on DVE: v = u*s_k; i = (v+M)-M;
        a = v-i (TT); b = |a| via bitwise-AND sign mask; then Sin(2pi*a)
        and Sin(-2pi*b + pi/2).
v*b_k folds into the decoder features (bf16 per-partition tensor_scalar), so
each score needs only 4 accumulating matmuls per frequency. The encoder
padding mask enters PSUM as a -1e30 seed via K=1 rank-1 bf16 matmuls emitted
during the input-DMA shadow; softmax runs without max-shift (bf16 exp, f32
accum_out row sums); the decoder mask folds into the 1/sum scale; output is
stored bf16 and upcast on host. Inputs arrive via two parallel-queue packed
bf16 DMAs -- [W1|encT] first since it feeds the critical uE->copy->chain
path, then [W2|decT|vb:dm] (the f32 vb/dm rides as raw bits, bitcast on
device, with explicit add_dep edges on its readers) -- plus a tiny mask-row
DMA. Exp/Sin ACT table sets are preloaded via dummy activations before the
data arrives; transposes/casts are host-side layout prep.
"""
import os
import sys

import numpy as np

if "/opt/trn_rl_repo" not in sys.path:
    sys.path.insert(0, "/opt/trn_rl_repo")

S, T, B, H = 512, 256, 8, 128
F = 4
OMEGA = np.array(
    [0.28378837870584145, 0.8460440611349613,
     1.5065032153851483, 2.6337314948021557], dtype=np.float64
)
BK = np.array(
    [1.2424371140304091, 0.31247076876828433,
     0.16201975442700692, 0.045413278208112536], dtype=np.float64
)
TWO_PI = float(2.0 * np.pi)
PI = float(np.pi)
HALF_PI = float(0.5 * np.pi)
NEG_BIG = -1.0e30

_CACHE = {}
LAST_EXEC_NS = None


def _try_install_trace_hook():
    """Best-effort NTFF profile hook for axon (used only when tracing)."""
    try:
        import contextlib
        import ctypes
        import types

        if "antenv.axon_hooks" in sys.modules:
            return
        lib = ctypes.CDLL("/opt/axon/libaxon_pjrt.so")
        if not hasattr(lib, "axon_start_nrt_profile"):
            return
        lib.axon_start_nrt_profile.argtypes = [
            ctypes.POINTER(ctypes.c_int64),
            ctypes.c_size_t,
        ]
        lib.axon_start_nrt_profile.restype = ctypes.c_int64
        lib.axon_stop_nrt_profile.argtypes = [ctypes.c_char_p]
        lib.axon_stop_nrt_profile.restype = ctypes.c_int64

        @contextlib.contextmanager
        def _hook(output_dir, device_ids):
            import jax

            jax.devices()
            if device_ids:
                ids = (ctypes.c_int64 * len(device_ids))(*device_ids)
                rc = lib.axon_start_nrt_profile(ids, len(device_ids))
            else:
                rc = lib.axon_start_nrt_profile(None, 0)
            if rc != 0:
                raise RuntimeError(f"axon_start_nrt_profile rc={rc}")
            try:
                yield
            finally:
                n = lib.axon_stop_nrt_profile(str(output_dir).encode())
                if n < 0:
                    raise RuntimeError(f"axon_stop_nrt_profile rc={n}")

        mod = types.ModuleType("antenv.axon_hooks")
        _h = _hook

        def set_axon_ntff_profile_hook(h):
            pass

        def get_axon_ntff_profile_hook():
            return _h

        mod.set_axon_ntff_profile_hook = set_axon_ntff_profile_hook
        mod.get_axon_ntff_profile_hook = get_axon_ntff_profile_hook
        sys.modules["antenv.axon_hooks"] = mod
        import antenv

        antenv.axon_hooks = mod
    except Exception:
        pass


def _build():
    if "nc" in _CACHE:
        return _CACHE["nc"]
    import concourse.bacc as bacc
    import concourse.tile as tile
    from concourse.tile import add_dep_helper
    import concourse.mybir as mybir

    F32 = mybir.dt.float32
    U32 = mybir.dt.uint32
    BF16 = mybir.dt.bfloat16
    FP16 = mybir.dt.float16
    AF = mybir.ActivationFunctionType
    AL = mybir.AluOpType

    SCAL = [float(w / (2.0 * np.pi)) for w in OMEGA]

    nc = bacc.Bacc("TRN2", target_bir_lowering=False, debug=False, num_devices=8)

    PK1C = (H + T) + 2 * (F + 2)
    pk2_d = nc.dram_tensor("pack2", [H, H + S], BF16, kind="ExternalInput")
    pk1_d = nc.dram_tensor("pack1", [H, PK1C], BF16, kind="ExternalInput")
    em_d = nc.dram_tensor("encmask", [1, S], BF16, kind="ExternalInput")
    out_d = nc.dram_tensor("out", [T, S], BF16, kind="ExternalOutput")

    with tile.TileContext(nc) as tc:
        with (
            tc.tile_pool(name="cst", bufs=1) as cst,
            tc.tile_pool(name="wrk", bufs=1) as wrk,
            tc.tile_pool(name="ps", bufs=1, space="PSUM") as psp,
        ):
            # ---- inputs via two parallel-queue DMAs: the e-side pack
            # ([W1|encT], feeds the critical uE->copy->chain path) rings
            # first, then [W2|decT|vb:dm-bits], then the tiny mask row ----
            with nc.named_scope("dma_in"):
                pk2_sb = cst.tile([H, H + S], BF16)
                nc.sync.dma_start(pk2_sb[:], pk2_d[:])
                pk1_sb = cst.tile([H, PK1C], BF16)
                pk_dma = nc.sync.dma_start(pk1_sb[:], pk1_d[:])
                em_sb = cst.tile([1, S], BF16)
                nc.sync.dma_start(em_sb[:], em_d[:])

            p1 = pk1_sb[:, 0:H + T]
            p2 = pk2_sb[:]
            p3 = pk1_sb[:, H + T:PK1C].bitcast(F32)

            ones_sb = cst.tile([1, H], BF16)
            nc.gpsimd.memset(ones_sb[:], 1.0)
            hp_sb = cst.tile([128, 1], F32)
            nc.gpsimd.memset(hp_sb[:], HALF_PI)

            # Preload the Exp table set into the second table slot while
            # ACT is idle, so the softmax needs no mid-stream table load.
            scr = wrk.tile([128, 1], F32, name="scr")
            nc.scalar.activation(scr[:], hp_sb[:], AF.Exp)
            scr2 = wrk.tile([128, 1], F32, name="scr2")
            nc.scalar.activation(scr2[:], hp_sb[:], AF.Sin)

            # ---- u in PSUM as TWO tensors (bank-disjoint) so the two
            # PSUM->SBUF copies (DVE: e, ACT: d) run in parallel; Tile
            # serializes cross-engine access within one PSUM tensor ----
            uE_ps = psp.tile([128, S], F32, tag="upsE")
            uD_ps = psp.tile([128, T], F32, tag="upsD")
            with nc.named_scope("u_mm"):
                nc.tensor.matmul(
                    uE_ps[:], p2[:, 0:H], p2[:, H:], start=True, stop=True)
                nc.tensor.matmul(
                    uD_ps[:], p1[:, 0:H], p1[:, H:], start=True, stop=True)
            u_sb = wrk.tile([128, 768], F32, name="u_sb")
            uA = u_sb[:, 0:S + T]
            with nc.named_scope("u_copy"):
                nc.vector.tensor_scalar_mul(u_sb[:, 0:S], uE_ps[:], 1.0)
                nc.scalar.copy(u_sb[:, S:], uD_ps[:])

            sc = []
            # score PSUM seeded with -1e30 encoder mask
            for tb in range(2):
                sc_tile = psp.tile([128, S], F32, tag=f"sc{tb}")
                sc.append(sc_tile)
                with nc.named_scope(f"mask_{tb}"):
                    nc.tensor.matmul(
                        sc_tile[:], ones_sb[:], em_sb[:],
                        start=True, stop=False, skip_group_check=True,
                    )


            W = S + T  # 768: all feature tiles are [e(512) | d(256)]
            M32 = float(1.5 * 2**23)

            def folds(k, fS, fC):
                """v*b_k folds into d-side features -> new [128,T] bf16 tiles.
                Explicit dep on the pack DMA: p3 is a bitcast view and must
                not race ahead of the transfer."""
                with nc.named_scope(f"vfold_{k}"):
                    fSdv = wrk.tile([128, T], BF16, name=f"fSdv{k}")
                    i1 = nc.vector.tensor_scalar_mul(
                        fSdv[:], fS[:, S:], p3[:, k:k + 1])
                    fCdv = wrk.tile([128, T], BF16, name=f"fCdv{k}")
                    i2 = nc.vector.tensor_scalar_mul(
                        fCdv[:], fC[:, S:], p3[:, k:k + 1])
                add_dep_helper(i1.ins, pk_dma.ins, reason="p3 bitcast read after DMA")
                add_dep_helper(i2.ins, pk_dma.ins, reason="p3 bitcast read after DMA")
                return fSdv, fCdv

            def scores(k, fSdv, fCdv, fS, fC):
                with nc.named_scope(f"scores_{k}"):
                    last = k == F - 1
                    for tb in range(2):
                        dsl = slice(tb * 128, (tb + 1) * 128)
                        nc.tensor.matmul(
                            sc[tb][:], fSdv[:, dsl], fC[:, 0:S],
                            start=False, stop=False, skip_group_check=True,
                        )
                        nc.tensor.matmul(
                            sc[tb][:], fCdv[:, dsl], fS[:, 0:S],
                            start=False, stop=last, skip_group_check=True,
                        )

            # ---- ACT stream (all inputs from SBUF) ----
            with nc.named_scope("sin_0"):
                fS0 = wrk.tile([128, W], BF16, name="fS0")
                nc.scalar.activation(fS0[:], uA, AF.Sin, scale=float(OMEGA[0]))
                fC0 = wrk.tile([128, W], BF16, name="fC0")
                nc.scalar.activation(
                    fC0[:], uA, AF.Sin, bias=hp_sb[:], scale=float(OMEGA[0]))

            absu = wrk.tile([128, W], F32, name="absu")
            absu_i = nc.vector.tensor_scalar(
                absu[:].bitcast(U32), uA.bitcast(U32), 0x7FFFFFFF, None,
                AL.bitwise_and)

            with nc.named_scope("sin_1"):
                fS1 = wrk.tile([128, W], BF16, name="fS1")
                nc.scalar.activation(fS1[:], uA, AF.Sin, scale=float(OMEGA[1]))
                fC1 = wrk.tile([128, W], BF16, name="fC1")
                fC1_i = nc.scalar.activation(
                    fC1[:], absu[:], AF.Sin, bias=hp_sb[:],
                    scale=float(-OMEGA[1]))
            add_dep_helper(fC1_i.ins, absu_i.ins, reason="fC1 reads sign-masked absu")

            # ---- k=2,3: magic-constant range reduction (all DVE), with
            # folds interleaved so PE gets its lhs operands early; |x| via
            # sign-bit masking (single-source tensor_scalar, 2x mode) ----
            parts = {}
            for k in (2, 3):
                with nc.named_scope(f"red_{k}"):
                    vv = wrk.tile([128, W], F32, name=f"v{k}")
                    nc.vector.tensor_scalar_mul(vv[:], uA, SCAL[k])
                    ii = wrk.tile([128, W], F32, name=f"i{k}")
                    nc.vector.tensor_scalar(
                        ii[:], vv[:], M32, M32, AL.add, AL.subtract)
                    aa = wrk.tile([128, W], F32, name=f"a{k}")
                    nc.vector.tensor_tensor(aa[:], vv[:], ii[:], AL.subtract)
                    bb = wrk.tile([128, W], F32, name=f"b{k}")
                    bb_i = nc.vector.tensor_scalar(
                        bb[:].bitcast(U32), aa[:].bitcast(U32), 0x7FFFFFFF,
                        None, AL.bitwise_and)
                with nc.named_scope(f"sin_{k}"):
                    fSk = wrk.tile([128, W], BF16, name=f"fS{k}")
                    nc.scalar.activation(fSk[:], aa[:], AF.Sin, scale=TWO_PI)
                    fCk = wrk.tile([128, W], BF16, name=f"fC{k}")
                    fCk_i = nc.scalar.activation(
                        fCk[:], bb[:], AF.Sin, bias=hp_sb[:], scale=-TWO_PI)
                add_dep_helper(fCk_i.ins, bb_i.ins, reason="fC reads sign-masked b")
                parts[k] = (fSk, fCk)
                if k == 2:
                    fS0v, fC0v = folds(0, fS0, fC0)
                    scores(0, fS0v, fC0v, fS0, fC0)
            fS1v, fC1v = folds(1, fS1, fC1)
            scores(1, fS1v, fC1v, fS1, fC1)
            for k in (2, 3):
                fSk, fCk = parts[k]
                fSkv, fCkv = folds(k, fSk, fCk)
                scores(k, fSkv, fCkv, fSk, fCk)

            # ---- softmax + decoder-mask scale + store; the two row-block
            # chains are interleaved so tb1's accum/recip overlaps tb0's
            # scale+store ----
            ex, rs, fac, ot = {}, {}, {}, {}
            for tb in range(2):
                with nc.named_scope(f"exp_{tb}"):
                    ex[tb] = wrk.tile([128, S], BF16, name=f"ex{tb}")
                    rs[tb] = wrk.tile([128, 1], F32, name=f"rs{tb}")
                    nc.scalar.activation(
                        ex[tb][:], sc[tb][:], AF.Exp, accum_out=rs[tb][:])
            for tb in range(2):
                with nc.named_scope(f"scale_{tb}"):
                    ri = wrk.tile([128, 1], F32, name=f"ri{tb}")
                    nc.vector.reciprocal(ri[:], rs[tb][:])
                    fac[tb] = wrk.tile([128, 1], F32, name=f"fac{tb}")
                    fac_i = nc.vector.tensor_tensor(
                        fac[tb][:], ri[:], p3[:, F + tb:F + tb + 1],
                        mybir.AluOpType.mult)
                    add_dep_helper(fac_i.ins, pk_dma.ins, reason="p3 bitcast read")
                    ot[tb] = wrk.tile([128, S], BF16, name=f"ot{tb}")
                    nc.vector.tensor_scalar_mul(ot[tb][:], ex[tb][:], fac[tb][:])
                    nc.sync.dma_start(out_d[tb * 128:(tb + 1) * 128, :], ot[tb][:])

    nc.compile()
    _CACHE["nc"] = nc
    return nc


def kernel(encoder_output, decoder_output, W1, W2, v, enc_lens, dec_lens):
    global LAST_EXEC_NS
    from concourse.bass_utils import run_bass_kernel_spmd
    import ml_dtypes

    BF = ml_dtypes.bfloat16
    enc = np.asarray(encoder_output, dtype=np.float32)
    dec = np.asarray(decoder_output, dtype=np.float32)
    W1 = np.asarray(W1, dtype=np.float32)
    W2 = np.asarray(W2, dtype=np.float32)
    v = np.asarray(v, dtype=np.float32)
    enc_lens = np.asarray(enc_lens)
    dec_lens = np.asarray(dec_lens)

    vb = (v[:, None].astype(np.float64) * BK[None, :]).astype(np.float32)  # (H,F)

    in_maps = []
    for b in range(B):
        p1 = np.concatenate([W2, dec[:, b, :].T], axis=1).astype(BF)
        p2 = np.ascontiguousarray(
            np.concatenate([W1, enc[:, b, :].T], axis=1).astype(BF))
        dm = (np.arange(T) < int(dec_lens[b])).astype(np.float32)
        p3 = np.ascontiguousarray(
            np.concatenate([vb, dm.reshape(H, 2, order="F")], axis=1),
            dtype=np.float32)
        p3_bits = p3.view(np.uint16).view(BF)  # raw f32 bits as bf16 pairs
        pack1 = np.ascontiguousarray(np.concatenate([p1, p3_bits], axis=1))
        em = np.where(
            np.arange(S)[None, :] < int(enc_lens[b]), 0.0, NEG_BIG
        ).astype(BF)
        in_maps.append({"pack1": pack1, "pack2": p2, "encmask": em})

    trace = os.environ.get("KERNEL_TRACE", "0") == "1"
    if trace:
        _try_install_trace_hook()
    nc = _build()
    ncores = int(os.environ.get("KERNEL_CORES", str(B)))
    res = run_bass_kernel_spmd(nc, in_maps[:ncores], core_ids=list(range(ncores)), trace=trace)
    if trace:
        LAST_EXEC_NS = res.exec_time_ns
        _CACHE["last_res"] = res

    out = np.zeros((T, B, S), dtype=np.float32)
    for b in range(ncores):
        out[:, b, :] = np.asarray(res.results[b]["out"], dtype=np.float32)
    return out
